# revision 23
# baseline (speedup 1.0000x reference)
"""Trainium2 Bass kernel for nn_DiffSOCSImager_1024x2048 (8-core SPMD), v2.

Derivation (see v1 for the SOCS reduction): the mode matrix's SVD reduces
to a 64x64 Gram eigendecomposition; numerical rank is 24, but modes 16..23
are below the output's fp16 noise floor (validated in simulation: max rel
err 1.55e-3 with 16 modes == with 24), so 16 real 114x114 kernels suffice:
exactly one complex-packed FFT convolution per core.

Performance structure vs v1:
  * All full-image transpose passes run on the DMA XBAR
    (InstDmaTransposeAnt; out[:, j, :] = in[:, j*128:(j+1)*128].T with a
    3D out AP batches 128x128-tile transposes per instruction), not the
    PE array.  Free-dim layouts keep every XBAR input tile contiguous;
    the image ping-pongs between SBUF plane-pairs A and B.
  * The kernel-pair forward FFT exploits its 114x114 support: a tiny fp32
    contract-h stage, one small XBAR, one full contract-w stage.
  * fp16 data everywhere else, 1024-col moving operands, PSUM-column
    orders chosen (via strided matmul rhs, which is free) so most PSUM
    evacuations write contiguous SBUF.

Index split: h = 8*h1 + h2, w = 128*w1 + w2, kh = kh1 + 128*kh2,
kw = kw1 + 16*kw2.  Layouts (q = h2*16 + w1 "hw-combo", d = kw1*8 + kh2):
  mask DRAM:        [p=h1  | h2*2048 + w2*16 + w1]
  X1 after S1 (A):  [p=kh1 | w2*128 + q]
  X2 after T1 (B):  [p=q   | w2*128 + kh1]
  X3 after S3' (A): [p=d   | kh1*128 + w2]
  X4 after T2 (B):  [p=w2  | kh1*128 + d]
  F  after S4' (A): [p=kw2 | kw1*1024 + kh1*8 + kh2]   (mask spectrum)
  kernel spec (B):  same as F
  Z1 after I1 (A):  [p=w2  | kh1*128 + d]
  Z2 after T1' (B): [p=d   | kh1*128 + w2]
  Z3 after I2 (A):  [p=q   | w2*128 + kh1]
  Z4 after T2' (B): [p=kh1 | w2*128 + q]
  out:              [p=h1  | h2*2048 + w1*128 + w2]    (row-major)
"""

import sys
import numpy as np

if "/opt/trn_rl_repo" not in sys.path:
    sys.path.insert(0, "/opt/trn_rl_repo")

# ---------------- static problem config ----------------
H, W = 1024, 2048
LAM, NA, DX = 193.0, 0.85, 1.0
N_SOCS, N_SOURCE = 32, 64
FC = NA / LAM
PI = float(np.pi)
CROP, HS = 115, 57
NK = 16
N_CORES = 8
P = 128
FREE = 16384
NSUP = 114
SCL = 64.0

# fp16 const sets (all mats 128x128 = 128 cols):
#   fwd: SA[8]x{re,im} (16) | M34p x{re,im,imn} (3) | SB[16]x3 (48)
#        | Aw[16]x3 (48)  -> 115 mats
#   inv: IA[8]x3 (24) | M34i x3 (3) | IB[16]x3 (48) -> 75 mats
NC16_COLS = 115 * 128
NI16_COLS = 75 * 128
RE, IM, IMN = 0, 1, 2

F_SA = lambda h2, pl: (h2 * 2 + pl) * 128
F_M34 = lambda pl: (16 + pl) * 128
F_SB = lambda kw1, pl: (19 + kw1 * 3 + pl) * 128
F_AW = lambda kw1, pl: (67 + kw1 * 3 + pl) * 128
I_IA = lambda h2, pl: (h2 * 3 + pl) * 128
I_M34 = lambda pl: (24 + pl) * 128
I_IB = lambda kw1, pl: (27 + kw1 * 3 + pl) * 128

# fp32 consts: Ah[8]x{re,im} (rows 114..127 zero)
NC32_COLS = 16 * 128


# ---------------- host: SOCS kernels ----------------

def _compute_kernels(sigma_c):
    """NK real 114x114 SOCS kernels scaled by SCL*sqrt(alpha)/(H*W)."""
    kymax = int(np.ceil(FC * H * DX)) + 1
    kxmax = int(np.ceil(FC * W * DX)) + 1
    KY, KX = np.meshgrid(np.arange(-kymax, kymax + 1),
                         np.arange(-kxmax, kxmax + 1), indexing="ij")
    fy32 = (KY.astype(np.float64) / (H * DX)).astype(np.float32)
    fx32 = (KX.astype(np.float64) / (W * DX)).astype(np.float32)
    sel = np.hypot(fx32, fy32) <= np.float32(FC)
    kyS = KY[sel]
    kxS = KX[sel]

    r_max = np.clip(np.float32(sigma_c), 0.01, 0.9) * np.float32(FC)
    n_r = int(np.sqrt(N_SOURCE * 0.3)) + 1
    n_theta = int(N_SOURCE / n_r) + 1
    r = np.linspace(0.0, 1.0, n_r, dtype=np.float32) * r_max
    theta = np.linspace(0.0, 2.0 * PI, n_theta, dtype=np.float32)
    rr, tt = np.meshgrid(r, theta, indexing="xy")
    fs = np.stack([(rr * np.cos(tt)).ravel(), (rr * np.sin(tt)).ravel()],
                  axis=1)[:N_SOURCE].astype(np.float32)

    fyS = (kyS / float(H * DX)).astype(np.float32)
    fxS = (kxS / float(W * DX)).astype(np.float32)
    cols = []
    for fp in fs:
        f1 = np.hypot(fxS + np.float32(fp[0] / 2), fyS + np.float32(fp[1] / 2))
        f2 = np.hypot(fxS - np.float32(fp[0] / 2), fyS - np.float32(fp[1] / 2))
        cols.append(((f1 <= np.float32(FC)) & (f2 <= np.float32(FC)))
                    .astype(np.float64))
    MS = np.stack(cols, axis=1)
    G = MS.T @ MS
    w_, V_ = np.linalg.eigh(G)
    idx = np.argsort(w_)[::-1]
    w_ = np.maximum(w_[idx], 0.0)
    V_ = V_[:, idx]
    keep = [k for k in range(min(NK, N_SOCS)) if w_[k] > 1e-9 * w_[0]]
    alpha = w_[keep]
    US = MS @ V_[:, keep] / np.sqrt(alpha)

    dy = np.arange(NSUP) - HS
    Ay = np.exp(2j * PI * np.outer(dy, kyS) / H) * ((-1.0) ** dy)[:, None]
    Ax = np.exp(2j * PI * np.outer(dy, kxS) / W) * ((-1.0) ** dy)[:, None]
    kerns = np.einsum("ys,sk,xs->kyx", Ay, US, Ax, optimize=True).real
    return kerns * (SCL * np.sqrt(alpha)[:, None, None] / (H * W))


# ---------------- host: stationaries ----------------

def _pack_consts():
    h1 = np.arange(128)[:, None]
    k1 = np.arange(128)[None, :]
    SA = [np.exp(-2j * PI * (h1 * k1 / 128.0 + h2 * k1 / 1024.0))
          for h2 in range(8)]
    a = (np.arange(128) // 8)[:, None]
    b = (np.arange(128) % 8)[:, None]
    c = (np.arange(128) // 8)[None, :]
    d = (np.arange(128) % 8)[None, :]
    M34 = np.exp(-2j * PI * (a * c / 16.0 + b * d / 8.0))
    # permutation: new index q = h2*16 + w1  <->  old index w1*8 + h2
    qperm = np.array([(q % 16) * 8 + q // 16 for q in range(128)])
    M34p = M34[qperm, :]      # fwd: rows = q-order, cols = (kw1*8+kh2)
    M34i = M34[:, qperm]      # inv: rows = (kw1*8+kh2), cols = q-order
    w2 = np.arange(128)[:, None]
    kw2 = np.arange(128)[None, :]
    SB = [np.exp(-2j * PI * (w2 * kw2 / 128.0 + w2 * kw1 / 2048.0))
          for kw1 in range(16)]
    IA = [np.conj(m).T for m in SA]
    IB = [np.conj(m).T for m in SB]

    dy = np.arange(NSUP) - HS
    Ah = []
    for bb in range(8):
        kh = np.arange(128)[None, :] + 128 * bb
        Ah.append(np.exp(-2j * PI * dy[:, None] * kh / 1024.0))
    Aw = []
    for kw1 in range(16):
        kw = kw1 + 16 * np.arange(128)[None, :]
        Aw.append(np.exp(-2j * PI * dy[:, None] * kw / 2048.0))

    def planes(m, n_planes):
        m64 = np.asarray(m, np.complex128)
        out = []
        for pm in (m64.real, m64.imag, -m64.imag)[:n_planes]:
            pm = np.asarray(pm, np.float32)
            if pm.shape[0] < 128:
                z = np.zeros((128, 128), np.float32)
                z[:pm.shape[0]] = pm
                pm = z
            out.append(pm)
        return out

    fwd = ([p for m in SA for p in planes(m, 2)]
           + planes(M34p, 3)
           + [p for m in SB for p in planes(m, 3)]
           + [p for m in Aw for p in planes(m, 3)])
    inv = ([p for m in IA for p in planes(m, 3)]
           + planes(M34i, 3)
           + [p for m in IB for p in planes(m, 3)])
    cf16 = np.concatenate(fwd, axis=1).astype(np.float16)
    ci16 = np.concatenate(inv, axis=1).astype(np.float16)
    assert cf16.shape[1] == NC16_COLS and ci16.shape[1] == NI16_COLS

    c32 = np.concatenate([p for m in Ah for p in planes(m, 2)],
                         axis=1).astype(np.float32)
    assert c32.shape[1] == NC32_COLS
    return cf16, ci16, c32


# ---------------- host: input packing ----------------

def _mask_layout(mask):
    """[p=h1 | h2*2048 + w2*16 + w1] fp16, rolled and scaled by 1/SCL."""
    m_u = np.roll(np.asarray(mask, np.float32), (-H // 2, -W // 2), axis=(0, 1))
    m_u = m_u * np.float32(1.0 / SCL)
    m4 = m_u.reshape(P, 8, 16, 128).transpose(0, 1, 3, 2)  # [h1,h2,w2,w1]
    return np.ascontiguousarray(m4.reshape(P, FREE)).astype(np.float16)


def _kern_pack(kp):
    """complex (114,114) crop-indexed pair -> [128, 342] fp32
    layout [-im | re | im] (step1 rhs trick), pad rows zero."""
    q = np.zeros((P, 3 * NSUP), np.float32)
    q[:NSUP, 0:NSUP] = -kp.imag
    q[:NSUP, NSUP:2 * NSUP] = kp.real
    q[:NSUP, 2 * NSUP:3 * NSUP] = kp.imag
    return q


# ---------------- bass program ----------------

_NC_CACHE = {}


def _build_nc(num_devices=N_CORES):
    import concourse.bacc as bacc
    import concourse.mybir as mybir
    import concourse.tile as tile

    dt = mybir.dt.float32
    db = mybir.dt.float16
    nc = bacc.Bacc("TRN2", target_bir_lowering=False, debug=False,
                   num_devices=num_devices)
    mask_d = nc.dram_tensor("mask_l", [P, FREE], db, kind="ExternalInput")
    kq_d = nc.dram_tensor("kq", [P, 3 * NSUP], dt, kind="ExternalInput")
    cf16_d = nc.dram_tensor("cf16", [P, NC16_COLS], db, kind="ExternalInput")
    ci16_d = nc.dram_tensor("ci16", [P, NI16_COLS], db, kind="ExternalInput")
    c32_d = nc.dram_tensor("c32", [P, NC32_COLS], dt, kind="ExternalInput")
    out_d = nc.dram_tensor("acc_out", [P, FREE], dt, kind="ExternalOutput")

    with tile.TileContext(nc) as tc:
        with (
            tc.tile_pool(name="img", bufs=1) as img_pool,
            tc.tile_pool(name="consts", bufs=1) as const_pool,
            tc.tile_pool(name="ksm", bufs=1) as ksm_pool,
            tc.tile_pool(name="mstream", bufs=1) as ms_pool,
            tc.tile_pool(name="acc", bufs=2) as acc_pool,
            tc.tile_pool(name="tmp", bufs=8) as tmp_pool,
            tc.tile_pool(name="t32", bufs=3) as t32_pool,
            tc.tile_pool(name="ps", bufs=8, space="PSUM") as ps_pool,
        ):
            xr = img_pool.tile([P, FREE], db, tag="xr")
            xi = img_pool.tile([P, FREE], db, tag="xi")
            yr = img_pool.tile([P, FREE], db, tag="yr")
            yi = img_pool.tile([P, FREE], db, tag="yi")
            c16 = const_pool.tile([P, NC16_COLS], db, tag="c16")
            c32 = const_pool.tile([P, NC32_COLS], dt, tag="c32")
            kt = ksm_pool.tile([P, 3 * NSUP], dt, tag="kt")
            tk1 = ksm_pool.tile([P, 2 * 8 * 128], db, tag="tk1")
            tk2 = ksm_pool.tile([P, 2 * 8 * 128], db, tag="tk2")

            A = (xr, xi)
            B = (yr, yi)

            def C16(off):
                return c16[:, off:off + 128]

            def C32(off):
                return c32[:, off:off + 128]

            _cp = [0]

            def copy_out(dst, src):
                # PSUM evacuation: only VectorE/ScalarE may read PSUM
                i = _cp[0] = _cp[0] + 1
                eng = (nc.vector.tensor_copy, nc.scalar.copy)[i % 2]
                eng(dst, src)

            nc.sync.dma_start(c16[:, 0:16 * 128], cf16_d.ap()[:, 0:16 * 128])
            nc.sync.dma_start(c16[:, 16 * 128:NC16_COLS],
                              cf16_d.ap()[:, 16 * 128:NC16_COLS])
            nc.sync.dma_start(c32[:], c32_d.ap())
            nc.sync.dma_start(kt[:], kq_d.ap())
            nc.gpsimd.memset(tk1[:], 0.0)

            # ---- S1: contract h1 per h2 (SA fp16) ----
            # mask chunk (h2, cw): cols (w2 in {64cw..}, w1:16), contiguous
            # dst X1 (A): free = w2*128 + h2*16 + w1: 32B-contig w1-runs
            xrv = xr[:].rearrange("p (w2 h2 w1) -> p w2 h2 w1",
                                  w2=128, h2=8, w1=16)
            xiv = xi[:].rearrange("p (w2 h2 w1) -> p w2 h2 w1",
                                  w2=128, h2=8, w1=16)
            for cw in range(2):
                for h2 in range(8):
                    mt = ms_pool.tile([P, 1024], db, tag="ms")
                    nc.sync.dma_start(
                        mt[:], mask_d.ap()[:, h2 * 2048 + cw * 1024:
                                           h2 * 2048 + (cw + 1) * 1024])
                    ps4 = [ps_pool.tile([P, 512], dt, tag="ps",
                                        name=f"s1{i}") for i in range(4)]
                    for h in range(2):
                        nc.tensor.matmul(ps4[h][:], C16(F_SA(h2, RE)),
                                         mt[:, h * 512:(h + 1) * 512],
                                         start=True, stop=True)
                    for h in range(2):
                        nc.tensor.matmul(ps4[2 + h][:], C16(F_SA(h2, IM)),
                                         mt[:, h * 512:(h + 1) * 512],
                                         start=True, stop=True)
                    for h in range(2):
                        psr = ps4[h][:].rearrange("p (a b) -> p a b",
                                                  a=32, b=16)
                        psi = ps4[2 + h][:].rearrange("p (a b) -> p a b",
                                                      a=32, b=16)
                        w0 = 64 * cw + 32 * h
                        copy_out(xrv[:, w0:w0 + 32, h2, :], psr)
                        copy_out(xiv[:, w0:w0 + 32, h2, :], psi)

            # ---- XBAR pass helper: per-128x128-tile transposes,
            # re plane on the SP queue, im plane on the Act queue ----
            def xbar_pass(src, dst, n_sub=4):
                # interleave planes so dependent stages unblock after ~2 subs
                step = 128 // n_sub
                for s in range(n_sub):
                    for pl in range(2):
                        dv = dst[pl][:].rearrange("p (j i) -> p j i",
                                                  j=128, i=128)
                        j0, j1 = s * step, (s + 1) * step
                        nc.sync.dma_start(
                            dv[:, j0:j1, :],
                            src[pl][:, j0 * 128:j1 * 128],
                            transpose=True)

            # ---- T1: A -> B : [kh1, q]@w2 -> [q, kh1]@w2 ----
            xbar_pass(A, B)

            # ---- generic full complex stage: 16 groups x 2 halves of 512,
            # both halves of a group share the group's stationaries ----
            def full_stage(src, dst, mats, rhs_of, dst_of, conj=False,
                           cp=None):
                cp = cp or copy_out
                for g in range(16):
                    mre, mim, mimn = mats(g)
                    if conj:
                        mim, mimn = mimn, mim
                    rre = [rhs_of(src[0], g, h) for h in range(2)]
                    rim = [rhs_of(src[1], g, h) for h in range(2)]
                    pre = [ps_pool.tile([P, 512], dt, tag="ps",
                                        name=f"fre{h}") for h in range(2)]
                    pim = [ps_pool.tile([P, 512], dt, tag="ps",
                                        name=f"fim{h}") for h in range(2)]
                    for h in range(2):
                        nc.tensor.matmul(pre[h][:], C16(mre), rre[h],
                                         start=True, stop=False)
                    for h in range(2):
                        nc.tensor.matmul(pim[h][:], C16(mre), rim[h],
                                         start=True, stop=False)
                    for h in range(2):
                        nc.tensor.matmul(pre[h][:], C16(mimn), rim[h],
                                         start=False, stop=True)
                    for h in range(2):
                        nc.tensor.matmul(pim[h][:], C16(mim), rre[h],
                                         start=False, stop=True)
                    for h in range(2):
                        cp(dst_of(dst[0], g, h), pre[h][:])
                        cp(dst_of(dst[1], g, h), pim[h][:])

            # S3'/I2-style: rhs contiguous (outer:4, inner:128); the
            # transposing scatter happens in the PSUM evacuation instead
            def contig_rhs(pap, g, h):
                off = g * 1024 + h * 512
                return pap[:, off:off + 512]

            def swap_dst(pap, g, h):
                # psum cols (a:4, b:128) -> dst free = b*128 + (4cc + a)
                v = pap[:].rearrange("p (b a) -> p b a", b=128, a=128)
                cc = 2 * g + h
                return v[:, :, 4 * cc:4 * cc + 4].transpose([0, 2, 1])

            contig_dst = contig_rhs

            # S4'/step2/I1-style: per-kw1 (kh1:64-half, kh2:8) slices
            def dkq_rhs(pap, kw1, h):
                v = pap[:].rearrange("p (k q d) -> p k q d", k=128, q=16, d=8)
                return v[:, 64 * h:64 * h + 64, kw1, :]

            dkq_dst = dkq_rhs

            # ---- S3': contract q via M34p : B -> A ----
            full_stage(B, A, lambda cc: (F_M34(RE), F_M34(IM), F_M34(IMN)),
                       contig_rhs, swap_dst)

            # ---- T2: A -> B : [d, w2]@kh1 -> [w2, d]@kh1 ----
            xbar_pass(A, B, n_sub=2)

            # ---- S4': contract w2 per kw1 (SB) : B -> A (mask spectrum)
            # single-chunk groups (kw1, h) so S4' starts after half of T2 ----
            def full_stage1(src, dst, mats, rhs_of, dst_of):
                for h in range(2):
                    for g in range(16):
                        mre, mim, mimn = mats(g)
                        rre = rhs_of(src[0], g, h)
                        rim = rhs_of(src[1], g, h)
                        pre = ps_pool.tile([P, 512], dt, tag="ps", name="g1re")
                        pim = ps_pool.tile([P, 512], dt, tag="ps", name="g1im")
                        nc.tensor.matmul(pre[:], C16(mre), rre,
                                         start=True, stop=False)
                        nc.tensor.matmul(pim[:], C16(mre), rim,
                                         start=True, stop=False)
                        nc.tensor.matmul(pre[:], C16(mimn), rim,
                                         start=False, stop=True)
                        nc.tensor.matmul(pim[:], C16(mim), rre,
                                         start=False, stop=True)
                        copy_out(dst_of(dst[0], g, h), pre[:])
                        copy_out(dst_of(dst[1], g, h), pim[:])

            full_stage1(B, A, lambda kw1: (F_SB(kw1, RE), F_SB(kw1, IM),
                                           F_SB(kw1, IMN)),
                        dkq_rhs, contig_dst)

            # ---- kernel fwd FFT step1 (fp32, contract h) ----
            for bb in range(8):
                ps = ps_pool.tile([P, 512], dt, tag="ps", name="k1")
                nc.tensor.matmul(ps[:, 0:2 * NSUP], C32((bb * 2 + RE) * 128),
                                 kt[:, NSUP:3 * NSUP], start=True, stop=False)
                nc.tensor.matmul(ps[:, 0:2 * NSUP], C32((bb * 2 + IM) * 128),
                                 kt[:, 0:2 * NSUP], start=False, stop=True)
                copy_out(tk1[:, bb * 128:bb * 128 + NSUP], ps[:, 0:NSUP])
                copy_out(tk1[:, (8 + bb) * 128:(8 + bb) * 128 + NSUP],
                         ps[:, NSUP:2 * NSUP])

            # Tk xbar: [kh1, (pl,b)*128+w] -> tk2 [w | (pl,b)*128 + kh1]
            tk2v = tk2[:].rearrange("p (j i) -> p j i", j=16, i=128)
            nc.sync.dma_start(tk2v[:, :, :], tk1[:, :], transpose=True)

            # ---- step2: contract w per kw1 (Aw) -> B (kernel spectrum) ----
            # rhs contiguous tk2 plane halves; psum (kh2r:4, kh1:128)
            # scattered into the F layout by the evacuation
            def k2_dst(pap, kw1, h):
                v = pap[:].rearrange("p (q k d) -> p q k d", q=16, k=128, d=8)
                return v[:, kw1, :, 4 * h:4 * h + 4].transpose([0, 2, 1])

            def s2_copy_out(dst, src_):
                i = _cp[0] = _cp[0] + 1
                eng = (nc.scalar.copy, nc.scalar.copy,
                       nc.vector.tensor_copy)[i % 3]
                eng(dst, src_)

            full_stage((0, 1), B,
                       lambda kw1: (F_AW(kw1, RE), F_AW(kw1, IM),
                                    F_AW(kw1, IMN)),
                       lambda pl, g, h: tk2[:, pl * 1024 + h * 512:
                                            pl * 1024 + (h + 1) * 512],
                       k2_dst, cp=s2_copy_out)

            # ---- swap consts to inv set, split so I1 (IB region) does
            # not wait on the Aw-tail stationary reads of step2 ----
            nc.sync.dma_start(c16[:, 0:27 * 128], ci16_d.ap()[:, 0:27 * 128])
            nc.sync.dma_start(c16[:, 27 * 128:NI16_COLS],
                              ci16_d.ap()[:, 27 * 128:NI16_COLS])

            # ---- product: B = A * B (complex fp16, in place) ----
            for cc in range(16):
                sl = slice(cc * 1024, (cc + 1) * 1024)
                t0 = tmp_pool.tile([P, 1024], db, tag="tp")
                t1 = tmp_pool.tile([P, 1024], db, tag="tp")
                t2 = tmp_pool.tile([P, 1024], db, tag="tp")
                t3 = tmp_pool.tile([P, 1024], db, tag="tp")
                # gpsimd TENSOR_TENSOR is ~2.4x slower than DVE: 2 ops vs 4,
                # and its ops issue first so they overlap the DVE muls
                nc.gpsimd.tensor_mul(t1[:], xi[:, sl], yi[:, sl])
                nc.gpsimd.tensor_mul(t3[:], xi[:, sl], yr[:, sl])
                nc.vector.tensor_mul(t0[:], xr[:, sl], yr[:, sl])
                nc.vector.tensor_mul(t2[:], xr[:, sl], yi[:, sl])
                nc.vector.tensor_sub(yr[:, sl], t0[:], t1[:])
                nc.vector.tensor_add(yi[:, sl], t2[:], t3[:])

            # ---- I1: contract kw2 per kw1 (IB) : B -> A ----
            # evacs biased to ScalarE: VectorE is saturated by the product
            _cp_sav = _cp[0]
            _cp[0] = 1  # start so scalar gets first pick

            def scalar_copy_out(dst, src_):
                i = _cp[0] = _cp[0] + 1
                eng = (nc.scalar.copy, nc.scalar.copy, nc.scalar.copy,
                       nc.vector.tensor_copy)[i % 4]
                eng(dst, src_)

            full_stage(B, A, lambda kw1: (I_IB(kw1, RE), I_IB(kw1, IM),
                                          I_IB(kw1, IMN)),
                       lambda pap, g, h: contig_dst(pap, g, h),
                       dkq_dst, cp=scalar_copy_out)

            # ---- T1': A -> B : [w2, d]@kh1 -> [d, w2]@kh1 ----
            xbar_pass(A, B)

            # ---- I2: contract d via M34i (conj) : B -> A ----
            full_stage(B, A, lambda cc: (I_M34(RE), I_M34(IM), I_M34(IMN)),
                       contig_rhs, swap_dst, conj=True)

            # ---- T2': A -> B : [q, kh1]@w2 -> [kh1, q]@w2 ----
            xbar_pass(A, B, n_sub=2)

            # ---- I3: contract kh1 per h2 (IA), square, accumulate, out ----
            yrv = yr[:].rearrange("p (w2 h2 w1) -> p w2 h2 w1",
                                  w2=128, h2=8, w1=16)
            yiv = yi[:].rearrange("p (w2 h2 w1) -> p w2 h2 w1",
                                  w2=128, h2=8, w1=16)
            for h2 in range(8):
              for half in range(2):
                arow = acc_pool.tile([P, 1024], dt, tag="arow")
                arv = arow[:].rearrange("p (w1 w2) -> p w1 w2", w1=16, w2=64)
                for c in (2 * half, 2 * half + 1):
                    # rhs enum (w2r:32, w1:16): w1 innermost = 32B-contig runs
                    rre = yrv[:, 32 * c:32 * c + 32, h2, :]
                    rim = yiv[:, 32 * c:32 * c + 32, h2, :]
                    pre = ps_pool.tile([P, 512], dt, tag="ps", name="i3re")
                    pim = ps_pool.tile([P, 512], dt, tag="ps", name="i3im")
                    nc.tensor.matmul(pre[:], C16(I_IA(h2, RE)), rre,
                                     start=True, stop=False)
                    nc.tensor.matmul(pim[:], C16(I_IA(h2, RE)), rim,
                                     start=True, stop=False)
                    nc.tensor.matmul(pre[:], C16(I_IA(h2, IMN)), rim,
                                     start=False, stop=True)
                    nc.tensor.matmul(pim[:], C16(I_IA(h2, IM)), rre,
                                     start=False, stop=True)
                    # psum cols (w1:16, w2r:32); squares into contig temps
                    sre = t32_pool.tile([P, 512], dt, tag="sq")
                    sim_ = t32_pool.tile([P, 512], dt, tag="sq")
                    nc.scalar.square(sre[:], pre[:])
                    nc.scalar.square(sim_[:], pim[:])
                    srev = sre[:].rearrange("p (a b) -> p a b", a=32, b=16)
                    simv = sim_[:].rearrange("p (a b) -> p a b", a=32, b=16)
                    c0 = 32 * (c - 2 * half)
                    nc.vector.tensor_add(
                        arv[:, :, c0:c0 + 32].transpose([0, 2, 1]),
                        srev, simv)
                odv = out_d.ap()[:, h2 * 2048:(h2 + 1) * 2048].rearrange(
                    "p (w1 w2) -> p w1 w2", w1=16, w2=128)
                nc.sync.dma_start(odv[:, :, 64 * half:64 * half + 64],
                                  arow[:].rearrange("p (a b) -> p a b",
                                                    a=16, b=64))

    nc.compile()
    return nc


# ---------------- entry point ----------------

def _prepare_inputs(mask, sigma_c):
    mask = np.asarray(mask, np.float32)
    kerns = _compute_kernels(float(np.asarray(sigma_c)))
    assert len(kerns) == NK
    mask_l = _mask_layout(mask)
    cf16, ci16, c32 = _pack_consts()
    in_maps = []
    for c in range(N_CORES):
        kp = kerns[c] + 1j * kerns[c + 8]
        in_maps.append({
            "mask_l": mask_l,
            "kq": _kern_pack(kp),
            "cf16": cf16,
            "ci16": ci16,
            "c32": c32,
        })
    return in_maps


def _combine(results):
    acc = np.zeros((H, W), np.float64)
    for c in range(N_CORES):
        acc += results[c]["acc_out"].astype(np.float64).reshape(H, W)
    I = np.fft.fftshift(acc)
    return (I / I.max()).astype(np.float32)


def kernel(mask, sigma_c, defocus_z4):
    from concourse import bass_utils

    in_maps = _prepare_inputs(mask, sigma_c)
    if "nc" not in _NC_CACHE:
        _NC_CACHE["nc"] = _build_nc()
    nc = _NC_CACHE["nc"]
    res = bass_utils.run_bass_kernel_spmd(nc, in_maps,
                                          core_ids=list(range(N_CORES)))
    return _combine(res.results)


# revision 25
# speedup vs baseline: 1.0686x; 1.0686x over previous
"""Trainium2 Bass kernel for nn_DiffSOCSImager_1024x2048 (8-core SPMD), v2.

Derivation (see v1 for the SOCS reduction): the mode matrix's SVD reduces
to a 64x64 Gram eigendecomposition; numerical rank is 24, but modes 16..23
are below the output's fp16 noise floor (validated in simulation: max rel
err 1.55e-3 with 16 modes == with 24), so 16 real 114x114 kernels suffice:
exactly one complex-packed FFT convolution per core.

Performance structure vs v1:
  * All full-image transpose passes run on the DMA XBAR
    (InstDmaTransposeAnt; out[:, j, :] = in[:, j*128:(j+1)*128].T with a
    3D out AP batches 128x128-tile transposes per instruction), not the
    PE array.  Free-dim layouts keep every XBAR input tile contiguous;
    the image ping-pongs between SBUF plane-pairs A and B.
  * The kernel-pair forward FFT exploits its 114x114 support: a tiny fp32
    contract-h stage, one small XBAR, one full contract-w stage.
  * fp16 data everywhere else, 1024-col moving operands, PSUM-column
    orders chosen (via strided matmul rhs, which is free) so most PSUM
    evacuations write contiguous SBUF.

Index split: h = 8*h1 + h2, w = 128*w1 + w2, kh = kh1 + 128*kh2,
kw = kw1 + 16*kw2.  Layouts (q = h2*16 + w1 "hw-combo", d = kw1*8 + kh2):
  mask DRAM:        [p=h1  | h2*2048 + w2*16 + w1]
  X1 after S1 (A):  [p=kh1 | w2*128 + q]
  X2 after T1 (B):  [p=q   | w2*128 + kh1]
  X3 after S3' (A): [p=d   | kh1*128 + w2]
  X4 after T2 (B):  [p=w2  | kh1*128 + d]
  F  after S4' (A): [p=kw2 | kw1*1024 + kh1*8 + kh2]   (mask spectrum)
  kernel spec (B):  same as F
  Z1 after I1 (A):  [p=w2  | kh1*128 + d]
  Z2 after T1' (B): [p=d   | kh1*128 + w2]
  Z3 after I2 (A):  [p=q   | w2*128 + kh1]
  Z4 after T2' (B): [p=kh1 | w2*128 + q]
  out:              [p=h1  | h2*2048 + w1*128 + w2]    (row-major)
"""

import sys
import numpy as np

if "/opt/trn_rl_repo" not in sys.path:
    sys.path.insert(0, "/opt/trn_rl_repo")

# ---------------- static problem config ----------------
H, W = 1024, 2048
LAM, NA, DX = 193.0, 0.85, 1.0
N_SOCS, N_SOURCE = 32, 64
FC = NA / LAM
PI = float(np.pi)
CROP, HS = 115, 57
NK = 16
N_CORES = 8
P = 128
FREE = 16384
NSUP = 114
SCL = 64.0

# fp16 const sets (all mats 128x128 = 128 cols):
#   fwd: SA[8]x{re,im} (16) | M34p x{re,im,imn} (3) | SB[16]x3 (48)
#        | Aw[16]x3 (48)  -> 115 mats
#   inv: IA[8]x3 (24) | M34i x3 (3) | IB[16]x3 (48) -> 75 mats
NC16_COLS = 115 * 128
NI16_COLS = 75 * 128
RE, IM, IMN = 0, 1, 2

F_SA = lambda h2, pl: (h2 * 2 + pl) * 128
F_M34 = lambda pl: (16 + pl) * 128
F_SB = lambda kw1, pl: (19 + kw1 * 3 + pl) * 128
F_AW = lambda kw1, pl: (67 + kw1 * 3 + pl) * 128
I_IA = lambda h2, pl: (h2 * 3 + pl) * 128
I_M34 = lambda pl: (24 + pl) * 128
I_IB = lambda kw1, pl: (27 + kw1 * 3 + pl) * 128

# fp32 consts: Ah[8]x{re,im} (rows 114..127 zero)
NC32_COLS = 16 * 128


# ---------------- host: SOCS kernels ----------------

def _compute_kernels(sigma_c):
    """NK real 114x114 SOCS kernels scaled by SCL*sqrt(alpha)/(H*W)."""
    kymax = int(np.ceil(FC * H * DX)) + 1
    kxmax = int(np.ceil(FC * W * DX)) + 1
    KY, KX = np.meshgrid(np.arange(-kymax, kymax + 1),
                         np.arange(-kxmax, kxmax + 1), indexing="ij")
    fy32 = (KY.astype(np.float64) / (H * DX)).astype(np.float32)
    fx32 = (KX.astype(np.float64) / (W * DX)).astype(np.float32)
    sel = np.hypot(fx32, fy32) <= np.float32(FC)
    kyS = KY[sel]
    kxS = KX[sel]

    r_max = np.clip(np.float32(sigma_c), 0.01, 0.9) * np.float32(FC)
    n_r = int(np.sqrt(N_SOURCE * 0.3)) + 1
    n_theta = int(N_SOURCE / n_r) + 1
    r = np.linspace(0.0, 1.0, n_r, dtype=np.float32) * r_max
    theta = np.linspace(0.0, 2.0 * PI, n_theta, dtype=np.float32)
    rr, tt = np.meshgrid(r, theta, indexing="xy")
    fs = np.stack([(rr * np.cos(tt)).ravel(), (rr * np.sin(tt)).ravel()],
                  axis=1)[:N_SOURCE].astype(np.float32)

    fyS = (kyS / float(H * DX)).astype(np.float32)
    fxS = (kxS / float(W * DX)).astype(np.float32)
    cols = []
    for fp in fs:
        f1 = np.hypot(fxS + np.float32(fp[0] / 2), fyS + np.float32(fp[1] / 2))
        f2 = np.hypot(fxS - np.float32(fp[0] / 2), fyS - np.float32(fp[1] / 2))
        cols.append(((f1 <= np.float32(FC)) & (f2 <= np.float32(FC)))
                    .astype(np.float64))
    MS = np.stack(cols, axis=1)
    G = MS.T @ MS
    w_, V_ = np.linalg.eigh(G)
    idx = np.argsort(w_)[::-1]
    w_ = np.maximum(w_[idx], 0.0)
    V_ = V_[:, idx]
    keep = [k for k in range(min(NK, N_SOCS)) if w_[k] > 1e-9 * w_[0]]
    alpha = w_[keep]
    US = MS @ V_[:, keep] / np.sqrt(alpha)

    dy = np.arange(NSUP) - HS
    Ay = np.exp(2j * PI * np.outer(dy, kyS) / H) * ((-1.0) ** dy)[:, None]
    Ax = np.exp(2j * PI * np.outer(dy, kxS) / W) * ((-1.0) ** dy)[:, None]
    kerns = np.einsum("ys,sk,xs->kyx", Ay, US, Ax, optimize=True).real
    return kerns * (SCL * np.sqrt(alpha)[:, None, None] / (H * W))


# ---------------- host: stationaries ----------------

def _pack_consts():
    h1 = np.arange(128)[:, None]
    k1 = np.arange(128)[None, :]
    SA = [np.exp(-2j * PI * (h1 * k1 / 128.0 + h2 * k1 / 1024.0))
          for h2 in range(8)]
    a = (np.arange(128) // 8)[:, None]
    b = (np.arange(128) % 8)[:, None]
    c = (np.arange(128) // 8)[None, :]
    d = (np.arange(128) % 8)[None, :]
    M34 = np.exp(-2j * PI * (a * c / 16.0 + b * d / 8.0))
    # permutation: new index q = h2*16 + w1  <->  old index w1*8 + h2
    qperm = np.array([(q % 16) * 8 + q // 16 for q in range(128)])
    M34p = M34[qperm, :]      # fwd: rows = q-order, cols = (kw1*8+kh2)
    M34i = M34[:, qperm]      # inv: rows = (kw1*8+kh2), cols = q-order
    w2 = np.arange(128)[:, None]
    kw2 = np.arange(128)[None, :]
    SB = [np.exp(-2j * PI * (w2 * kw2 / 128.0 + w2 * kw1 / 2048.0))
          for kw1 in range(16)]
    IA = [np.conj(m).T for m in SA]
    IB = [np.conj(m).T for m in SB]

    dy = np.arange(NSUP) - HS
    Ah = []
    for bb in range(8):
        kh = np.arange(128)[None, :] + 128 * bb
        Ah.append(np.exp(-2j * PI * dy[:, None] * kh / 1024.0))
    Aw = []
    for kw1 in range(16):
        kw = kw1 + 16 * np.arange(128)[None, :]
        Aw.append(np.exp(-2j * PI * dy[:, None] * kw / 2048.0))

    def planes(m, n_planes):
        m64 = np.asarray(m, np.complex128)
        out = []
        for pm in (m64.real, m64.imag, -m64.imag)[:n_planes]:
            pm = np.asarray(pm, np.float32)
            if pm.shape[0] < 128:
                z = np.zeros((128, 128), np.float32)
                z[:pm.shape[0]] = pm
                pm = z
            out.append(pm)
        return out

    fwd = ([p for m in SA for p in planes(m, 2)]
           + planes(M34p, 3)
           + [p for m in SB for p in planes(m, 3)]
           + [p for m in Aw for p in planes(m, 3)])
    inv = ([p for m in IA for p in planes(m, 3)]
           + planes(M34i, 3)
           + [p for m in IB for p in planes(m, 3)])
    cf16 = np.concatenate(fwd, axis=1).astype(np.float16)
    ci16 = np.concatenate(inv, axis=1).astype(np.float16)
    assert cf16.shape[1] == NC16_COLS and ci16.shape[1] == NI16_COLS

    c32 = np.concatenate([p for m in Ah for p in planes(m, 2)],
                         axis=1).astype(np.float32)
    assert c32.shape[1] == NC32_COLS
    return cf16, ci16, c32


# ---------------- host: input packing ----------------

def _mask_layout(mask):
    """[p=h1 | h2*2048 + w2*16 + w1] fp16, rolled and scaled by 1/SCL."""
    m_u = np.roll(np.asarray(mask, np.float32), (-H // 2, -W // 2), axis=(0, 1))
    m_u = m_u * np.float32(1.0 / SCL)
    m4 = m_u.reshape(P, 8, 16, 128).transpose(0, 1, 3, 2)  # [h1,h2,w2,w1]
    return np.ascontiguousarray(m4.reshape(P, FREE)).astype(np.float16)


def _kern_pack(kp):
    """complex (114,114) crop-indexed pair -> [128, 342] fp32
    layout [-im | re | im] (step1 rhs trick), pad rows zero."""
    q = np.zeros((P, 3 * NSUP), np.float32)
    q[:NSUP, 0:NSUP] = -kp.imag
    q[:NSUP, NSUP:2 * NSUP] = kp.real
    q[:NSUP, 2 * NSUP:3 * NSUP] = kp.imag
    return q


# ---------------- bass program ----------------

_NC_CACHE = {}


def _build_nc(num_devices=N_CORES):
    import concourse.bacc as bacc
    import concourse.mybir as mybir
    import concourse.tile as tile

    dt = mybir.dt.float32
    db = mybir.dt.float16
    nc = bacc.Bacc("TRN2", target_bir_lowering=False, debug=False,
                   num_devices=num_devices)
    mask_d = nc.dram_tensor("mask_l", [P, FREE], db, kind="ExternalInput")
    kq_d = nc.dram_tensor("kq", [P, 3 * NSUP], dt, kind="ExternalInput")
    cf16_d = nc.dram_tensor("cf16", [P, NC16_COLS], db, kind="ExternalInput")
    ci16_d = nc.dram_tensor("ci16", [P, NI16_COLS], db, kind="ExternalInput")
    c32_d = nc.dram_tensor("c32", [P, NC32_COLS], dt, kind="ExternalInput")
    out_d = nc.dram_tensor("acc_out", [P, FREE], dt, kind="ExternalOutput")

    with tile.TileContext(nc) as tc:
        with (
            tc.tile_pool(name="img", bufs=1) as img_pool,
            tc.tile_pool(name="consts", bufs=1) as const_pool,
            tc.tile_pool(name="ksm", bufs=1) as ksm_pool,
            tc.tile_pool(name="mstream", bufs=2) as ms_pool,
            tc.tile_pool(name="acc", bufs=3) as acc_pool,
            tc.tile_pool(name="tmp", bufs=4) as tmp_pool,
            tc.tile_pool(name="t32", bufs=4) as t32_pool,
            tc.tile_pool(name="ps", bufs=8, space="PSUM") as ps_pool,
        ):
            xr = img_pool.tile([P, FREE], db, tag="xr")
            xi = img_pool.tile([P, FREE], db, tag="xi")
            yr = img_pool.tile([P, FREE], db, tag="yr")
            yi = img_pool.tile([P, FREE], db, tag="yi")
            c16 = const_pool.tile([P, NC16_COLS], db, tag="c16")
            c32 = const_pool.tile([P, NC32_COLS], dt, tag="c32")
            kt = ksm_pool.tile([P, 3 * NSUP], dt, tag="kt")
            tk1 = ksm_pool.tile([P, 2 * 8 * 128], db, tag="tk1")
            tk2 = ksm_pool.tile([P, 2 * 8 * 128], db, tag="tk2")

            A = (xr, xi)
            B = (yr, yi)

            def C16(off):
                return c16[:, off:off + 128]

            def C32(off):
                return c32[:, off:off + 128]

            _cp = [0]

            def copy_out(dst, src):
                # PSUM evacuation: only VectorE/ScalarE may read PSUM
                i = _cp[0] = _cp[0] + 1
                eng = (nc.vector.tensor_copy, nc.scalar.copy)[i % 2]
                eng(dst, src)

            nc.sync.dma_start(c16[:, 0:16 * 128], cf16_d.ap()[:, 0:16 * 128])
            nc.sync.dma_start(c16[:, 16 * 128:NC16_COLS],
                              cf16_d.ap()[:, 16 * 128:NC16_COLS])
            nc.sync.dma_start(c32[:], c32_d.ap())
            nc.sync.dma_start(kt[:], kq_d.ap())
            nc.gpsimd.memset(tk1[:], 0.0)

            # ---- S1: contract h1 per h2 (SA fp16) ----
            # mask chunk (h2, cw): cols (w2 in {64cw..}, w1:16), contiguous
            # dst X1 (A): free = w2*128 + h2*16 + w1: 32B-contig w1-runs
            xrv = xr[:].rearrange("p (w2 h2 w1) -> p w2 h2 w1",
                                  w2=128, h2=8, w1=16)
            xiv = xi[:].rearrange("p (w2 h2 w1) -> p w2 h2 w1",
                                  w2=128, h2=8, w1=16)
            for cw in range(2):
                for h2 in range(8):
                    mt = ms_pool.tile([P, 1024], db, tag="ms")
                    nc.sync.dma_start(
                        mt[:], mask_d.ap()[:, h2 * 2048 + cw * 1024:
                                           h2 * 2048 + (cw + 1) * 1024])
                    ps4 = [ps_pool.tile([P, 512], dt, tag="ps",
                                        name=f"s1{i}") for i in range(4)]
                    for h in range(2):
                        nc.tensor.matmul(ps4[h][:], C16(F_SA(h2, RE)),
                                         mt[:, h * 512:(h + 1) * 512],
                                         start=True, stop=True)
                    for h in range(2):
                        nc.tensor.matmul(ps4[2 + h][:], C16(F_SA(h2, IM)),
                                         mt[:, h * 512:(h + 1) * 512],
                                         start=True, stop=True)
                    for h in range(2):
                        psr = ps4[h][:].rearrange("p (a b) -> p a b",
                                                  a=32, b=16)
                        psi = ps4[2 + h][:].rearrange("p (a b) -> p a b",
                                                      a=32, b=16)
                        w0 = 64 * cw + 32 * h
                        copy_out(xrv[:, w0:w0 + 32, h2, :], psr)
                        copy_out(xiv[:, w0:w0 + 32, h2, :], psi)

            # ---- XBAR pass helper: per-128x128-tile transposes,
            # re plane on the SP queue, im plane on the Act queue ----
            def xbar_pass(src, dst, n_sub=4):
                # interleave planes so dependent stages unblock after ~2 subs
                step = 128 // n_sub
                for s in range(n_sub):
                    for pl in range(2):
                        dv = dst[pl][:].rearrange("p (j i) -> p j i",
                                                  j=128, i=128)
                        j0, j1 = s * step, (s + 1) * step
                        nc.sync.dma_start(
                            dv[:, j0:j1, :],
                            src[pl][:, j0 * 128:j1 * 128],
                            transpose=True)

            # ---- T1: A -> B : [kh1, q]@w2 -> [q, kh1]@w2 ----
            xbar_pass(A, B, n_sub=1)

            # ---- generic full complex stage: 16 groups x 2 halves of 512,
            # both halves of a group share the group's stationaries ----
            def full_stage(src, dst, mats, rhs_of, dst_of, conj=False,
                           cp=None):
                cp = cp or copy_out
                for g in range(16):
                    mre, mim, mimn = mats(g)
                    if conj:
                        mim, mimn = mimn, mim
                    rre = [rhs_of(src[0], g, h) for h in range(2)]
                    rim = [rhs_of(src[1], g, h) for h in range(2)]
                    pre = [ps_pool.tile([P, 512], dt, tag="ps",
                                        name=f"fre{h}") for h in range(2)]
                    pim = [ps_pool.tile([P, 512], dt, tag="ps",
                                        name=f"fim{h}") for h in range(2)]
                    for h in range(2):
                        nc.tensor.matmul(pre[h][:], C16(mre), rre[h],
                                         start=True, stop=False)
                    for h in range(2):
                        nc.tensor.matmul(pim[h][:], C16(mre), rim[h],
                                         start=True, stop=False)
                    for h in range(2):
                        nc.tensor.matmul(pre[h][:], C16(mimn), rim[h],
                                         start=False, stop=True)
                    for h in range(2):
                        nc.tensor.matmul(pim[h][:], C16(mim), rre[h],
                                         start=False, stop=True)
                    for h in range(2):
                        cp(dst_of(dst[0], g, h), pre[h][:])
                        cp(dst_of(dst[1], g, h), pim[h][:])

            # S3'/I2-style: rhs contiguous (outer:4, inner:128); the
            # transposing scatter happens in the PSUM evacuation instead
            def contig_rhs(pap, g, h):
                off = g * 1024 + h * 512
                return pap[:, off:off + 512]

            def swap_dst(pap, g, h):
                # psum cols (a:4, b:128) -> dst free = b*128 + (4cc + a)
                v = pap[:].rearrange("p (b a) -> p b a", b=128, a=128)
                cc = 2 * g + h
                return v[:, :, 4 * cc:4 * cc + 4].transpose([0, 2, 1])

            contig_dst = contig_rhs

            # S4'/step2/I1-style: per-kw1 (kh1:64-half, kh2:8) slices
            def dkq_rhs(pap, kw1, h):
                v = pap[:].rearrange("p (k q d) -> p k q d", k=128, q=16, d=8)
                return v[:, 64 * h:64 * h + 64, kw1, :]

            dkq_dst = dkq_rhs

            # ---- S3': contract q via M34p : B -> A ----
            full_stage(B, A, lambda cc: (F_M34(RE), F_M34(IM), F_M34(IMN)),
                       contig_rhs, swap_dst)

            # ---- T2: A -> B : [d, w2]@kh1 -> [w2, d]@kh1 ----
            xbar_pass(A, B)

            # ---- S4': contract w2 per kw1 (SB) : B -> A (mask spectrum)
            # single-chunk groups (kw1, h) so S4' starts after half of T2 ----
            def full_stage1(src, dst, mats, rhs_of, dst_of):
                for h in range(2):
                    for g in range(16):
                        mre, mim, mimn = mats(g)
                        rre = rhs_of(src[0], g, h)
                        rim = rhs_of(src[1], g, h)
                        pre = ps_pool.tile([P, 512], dt, tag="ps", name="g1re")
                        pim = ps_pool.tile([P, 512], dt, tag="ps", name="g1im")
                        nc.tensor.matmul(pre[:], C16(mre), rre,
                                         start=True, stop=False)
                        nc.tensor.matmul(pim[:], C16(mre), rim,
                                         start=True, stop=False)
                        nc.tensor.matmul(pre[:], C16(mimn), rim,
                                         start=False, stop=True)
                        nc.tensor.matmul(pim[:], C16(mim), rre,
                                         start=False, stop=True)
                        copy_out(dst_of(dst[0], g, h), pre[:])
                        copy_out(dst_of(dst[1], g, h), pim[:])

            full_stage1(B, A, lambda kw1: (F_SB(kw1, RE), F_SB(kw1, IM),
                                           F_SB(kw1, IMN)),
                        dkq_rhs, contig_dst)

            # ---- kernel fwd FFT step1 (fp32, contract h) ----
            for bb in range(8):
                ps = ps_pool.tile([P, 512], dt, tag="ps", name="k1")
                nc.tensor.matmul(ps[:, 0:2 * NSUP], C32((bb * 2 + RE) * 128),
                                 kt[:, NSUP:3 * NSUP], start=True, stop=False)
                nc.tensor.matmul(ps[:, 0:2 * NSUP], C32((bb * 2 + IM) * 128),
                                 kt[:, 0:2 * NSUP], start=False, stop=True)
                copy_out(tk1[:, bb * 128:bb * 128 + NSUP], ps[:, 0:NSUP])
                copy_out(tk1[:, (8 + bb) * 128:(8 + bb) * 128 + NSUP],
                         ps[:, NSUP:2 * NSUP])

            # Tk xbar: [kh1, (pl,b)*128+w] -> tk2 [w | (pl,b)*128 + kh1]
            tk2v = tk2[:].rearrange("p (j i) -> p j i", j=16, i=128)
            nc.sync.dma_start(tk2v[:, :, :], tk1[:, :], transpose=True)

            # ---- step2: contract w per kw1 (Aw) -> B (kernel spectrum) ----
            # rhs contiguous tk2 plane halves; psum (kh2r:4, kh1:128)
            # scattered into the F layout by the evacuation
            def k2_dst(pap, kw1, h):
                v = pap[:].rearrange("p (q k d) -> p q k d", q=16, k=128, d=8)
                return v[:, kw1, :, 4 * h:4 * h + 4].transpose([0, 2, 1])

            def s2_copy_out(dst, src_):
                i = _cp[0] = _cp[0] + 1
                eng = (nc.scalar.copy, nc.scalar.copy,
                       nc.vector.tensor_copy)[i % 3]
                eng(dst, src_)

            full_stage((0, 1), B,
                       lambda kw1: (F_AW(kw1, RE), F_AW(kw1, IM),
                                    F_AW(kw1, IMN)),
                       lambda pl, g, h: tk2[:, pl * 1024 + h * 512:
                                            pl * 1024 + (h + 1) * 512],
                       k2_dst, cp=s2_copy_out)

            # ---- swap consts to inv set, split so I1 (IB region) does
            # not wait on the Aw-tail stationary reads of step2 ----
            nc.sync.dma_start(c16[:, 0:27 * 128], ci16_d.ap()[:, 0:27 * 128])
            nc.sync.dma_start(c16[:, 27 * 128:NI16_COLS],
                              ci16_d.ap()[:, 27 * 128:NI16_COLS])

            # ---- product: B = A * B (complex fp16, in place) ----
            for cc in range(16):
                sl = slice(cc * 1024, (cc + 1) * 1024)
                t0 = tmp_pool.tile([P, 1024], db, tag="tp")
                t1 = tmp_pool.tile([P, 1024], db, tag="tp")
                t2 = tmp_pool.tile([P, 1024], db, tag="tp")
                t3 = tmp_pool.tile([P, 1024], db, tag="tp")
                # gpsimd TENSOR_TENSOR is ~2.4x slower than DVE: 2 ops vs 4,
                # issued first so they overlap the DVE muls
                nc.gpsimd.tensor_mul(t1[:], xi[:, sl], yi[:, sl])
                nc.gpsimd.tensor_mul(t3[:], xi[:, sl], yr[:, sl])
                nc.vector.tensor_mul(t0[:], xr[:, sl], yr[:, sl])
                nc.vector.tensor_mul(t2[:], xr[:, sl], yi[:, sl])
                nc.vector.tensor_sub(yr[:, sl], t0[:], t1[:])
                nc.vector.tensor_add(yi[:, sl], t2[:], t3[:])

            # ---- I1: contract kw2 per kw1 (IB) : B -> A ----
            # evacs biased to ScalarE: VectorE is saturated by the product
            _cp_sav = _cp[0]
            _cp[0] = 1  # start so scalar gets first pick

            def scalar_copy_out(dst, src_):
                i = _cp[0] = _cp[0] + 1
                eng = (nc.scalar.copy, nc.scalar.copy, nc.scalar.copy,
                       nc.vector.tensor_copy)[i % 4]
                eng(dst, src_)

            full_stage(B, A, lambda kw1: (I_IB(kw1, RE), I_IB(kw1, IM),
                                          I_IB(kw1, IMN)),
                       lambda pap, g, h: contig_dst(pap, g, h),
                       dkq_dst, cp=scalar_copy_out)

            # ---- T1': A -> B : [w2, d]@kh1 -> [d, w2]@kh1 ----
            xbar_pass(A, B, n_sub=1)

            # ---- I2: contract d via M34i (conj) : B -> A ----
            full_stage(B, A, lambda cc: (I_M34(RE), I_M34(IM), I_M34(IMN)),
                       contig_rhs, swap_dst, conj=True)

            # ---- T2': A -> B : [q, kh1]@w2 -> [kh1, q]@w2 ----
            xbar_pass(A, B)

            # ---- I3: contract kh1 per h2 (IA), square, accumulate, out ----
            yrv = yr[:].rearrange("p (w2 h2 w1) -> p w2 h2 w1",
                                  w2=128, h2=8, w1=16)
            yiv = yi[:].rearrange("p (w2 h2 w1) -> p w2 h2 w1",
                                  w2=128, h2=8, w1=16)
            for h2 in range(8):
              for half in range(2):
                arow = acc_pool.tile([P, 1024], dt, tag="arow")
                arv = arow[:].rearrange("p (w1 w2) -> p w1 w2", w1=16, w2=64)
                for c in (2 * half, 2 * half + 1):
                    # rhs enum (w2r:32, w1:16): w1 innermost = 32B-contig runs
                    rre = yrv[:, 32 * c:32 * c + 32, h2, :]
                    rim = yiv[:, 32 * c:32 * c + 32, h2, :]
                    pre = ps_pool.tile([P, 512], dt, tag="ps", name="i3re")
                    pim = ps_pool.tile([P, 512], dt, tag="ps", name="i3im")
                    nc.tensor.matmul(pre[:], C16(I_IA(h2, RE)), rre,
                                     start=True, stop=False)
                    nc.tensor.matmul(pim[:], C16(I_IA(h2, RE)), rim,
                                     start=True, stop=False)
                    nc.tensor.matmul(pre[:], C16(I_IA(h2, IMN)), rim,
                                     start=False, stop=True)
                    nc.tensor.matmul(pim[:], C16(I_IA(h2, IM)), rre,
                                     start=False, stop=True)
                    # psum cols (w1:16, w2r:32); squares into contig temps
                    sre = t32_pool.tile([P, 512], dt, tag="sq")
                    sim_ = t32_pool.tile([P, 512], dt, tag="sq")
                    nc.scalar.square(sre[:], pre[:])
                    nc.scalar.square(sim_[:], pim[:])
                    srev = sre[:].rearrange("p (a b) -> p a b", a=32, b=16)
                    simv = sim_[:].rearrange("p (a b) -> p a b", a=32, b=16)
                    c0 = 32 * (c - 2 * half)
                    nc.vector.tensor_add(
                        arv[:, :, c0:c0 + 32].transpose([0, 2, 1]),
                        srev, simv)
                odv = out_d.ap()[:, h2 * 2048:(h2 + 1) * 2048].rearrange(
                    "p (w1 w2) -> p w1 w2", w1=16, w2=128)
                nc.sync.dma_start(odv[:, :, 64 * half:64 * half + 64],
                                  arow[:].rearrange("p (a b) -> p a b",
                                                    a=16, b=64))

    nc.compile()
    return nc


# ---------------- entry point ----------------

def _prepare_inputs(mask, sigma_c):
    mask = np.asarray(mask, np.float32)
    kerns = _compute_kernels(float(np.asarray(sigma_c)))
    assert len(kerns) == NK
    mask_l = _mask_layout(mask)
    cf16, ci16, c32 = _pack_consts()
    in_maps = []
    for c in range(N_CORES):
        kp = kerns[c] + 1j * kerns[c + 8]
        in_maps.append({
            "mask_l": mask_l,
            "kq": _kern_pack(kp),
            "cf16": cf16,
            "ci16": ci16,
            "c32": c32,
        })
    return in_maps


def _combine(results):
    acc = np.zeros((H, W), np.float64)
    for c in range(N_CORES):
        acc += results[c]["acc_out"].astype(np.float64).reshape(H, W)
    I = np.fft.fftshift(acc)
    return (I / I.max()).astype(np.float32)


def kernel(mask, sigma_c, defocus_z4):
    from concourse import bass_utils

    in_maps = _prepare_inputs(mask, sigma_c)
    if "nc" not in _NC_CACHE:
        _NC_CACHE["nc"] = _build_nc()
    nc = _NC_CACHE["nc"]
    res = bass_utils.run_bass_kernel_spmd(nc, in_maps,
                                          core_ids=list(range(N_CORES)))
    return _combine(res.results)


# revision 27
# speedup vs baseline: 1.0811x; 1.0117x over previous
"""Trainium2 Bass kernel for nn_DiffSOCSImager_1024x2048 (8-core SPMD), v2.

Derivation (see v1 for the SOCS reduction): the mode matrix's SVD reduces
to a 64x64 Gram eigendecomposition; numerical rank is 24, but modes 16..23
are below the output's fp16 noise floor (validated in simulation: max rel
err 1.55e-3 with 16 modes == with 24), so 16 real 114x114 kernels suffice:
exactly one complex-packed FFT convolution per core.

Performance structure vs v1:
  * All full-image transpose passes run on the DMA XBAR
    (InstDmaTransposeAnt; out[:, j, :] = in[:, j*128:(j+1)*128].T with a
    3D out AP batches 128x128-tile transposes per instruction), not the
    PE array.  Free-dim layouts keep every XBAR input tile contiguous;
    the image ping-pongs between SBUF plane-pairs A and B.
  * The kernel-pair forward FFT exploits its 114x114 support: a tiny fp32
    contract-h stage, one small XBAR, one full contract-w stage.
  * fp16 data everywhere else, 1024-col moving operands, PSUM-column
    orders chosen (via strided matmul rhs, which is free) so most PSUM
    evacuations write contiguous SBUF.

Index split: h = 8*h1 + h2, w = 128*w1 + w2, kh = kh1 + 128*kh2,
kw = kw1 + 16*kw2.  Layouts (q = h2*16 + w1 "hw-combo", d = kw1*8 + kh2):
  mask DRAM:        [p=h1  | h2*2048 + w2*16 + w1]
  X1 after S1 (A):  [p=kh1 | w2*128 + q]
  X2 after T1 (B):  [p=q   | w2*128 + kh1]
  X3 after S3' (A): [p=d   | kh1*128 + w2]
  X4 after T2 (B):  [p=w2  | kh1*128 + d]
  F  after S4' (A): [p=kw2 | kw1*1024 + kh1*8 + kh2]   (mask spectrum)
  kernel spec (B):  same as F
  Z1 after I1 (A):  [p=w2  | kh1*128 + d]
  Z2 after T1' (B): [p=d   | kh1*128 + w2]
  Z3 after I2 (A):  [p=q   | w2*128 + kh1]
  Z4 after T2' (B): [p=kh1 | w2*128 + q]
  out:              [p=h1  | h2*2048 + w1*128 + w2]    (row-major)
"""

import sys
import numpy as np

if "/opt/trn_rl_repo" not in sys.path:
    sys.path.insert(0, "/opt/trn_rl_repo")

# ---------------- static problem config ----------------
H, W = 1024, 2048
LAM, NA, DX = 193.0, 0.85, 1.0
N_SOCS, N_SOURCE = 32, 64
FC = NA / LAM
PI = float(np.pi)
CROP, HS = 115, 57
NK = 16
N_CORES = 8
P = 128
FREE = 16384
NSUP = 114
SCL = 64.0

# fp16 const sets (all mats 128x128 = 128 cols):
#   fwd: SA[8]x{re,im} (16) | M34p x{re,im,imn} (3) | SB[16]x3 (48)
#        | Aw[16]x3 (48)  -> 115 mats
#   inv: IA[8]x3 (24) | M34i x3 (3) | IB[16]x3 (48) -> 75 mats
NC16_COLS = 115 * 128
NI16_COLS = 75 * 128
RE, IM, IMN = 0, 1, 2

F_SA = lambda h2, pl: (h2 * 2 + pl) * 128
F_M34 = lambda pl: (16 + pl) * 128
F_SB = lambda kw1, pl: (19 + kw1 * 3 + pl) * 128
F_AW = lambda kw1, pl: (67 + kw1 * 3 + pl) * 128
I_IA = lambda h2, pl: (h2 * 3 + pl) * 128
I_M34 = lambda pl: (24 + pl) * 128
I_IB = lambda kw1, pl: (27 + kw1 * 3 + pl) * 128

# fp32 consts: Ah[8]x{re,im} (rows 114..127 zero)
NC32_COLS = 16 * 128


# ---------------- host: SOCS kernels ----------------

def _compute_kernels(sigma_c):
    """NK real 114x114 SOCS kernels scaled by SCL*sqrt(alpha)/(H*W)."""
    kymax = int(np.ceil(FC * H * DX)) + 1
    kxmax = int(np.ceil(FC * W * DX)) + 1
    KY, KX = np.meshgrid(np.arange(-kymax, kymax + 1),
                         np.arange(-kxmax, kxmax + 1), indexing="ij")
    fy32 = (KY.astype(np.float64) / (H * DX)).astype(np.float32)
    fx32 = (KX.astype(np.float64) / (W * DX)).astype(np.float32)
    sel = np.hypot(fx32, fy32) <= np.float32(FC)
    kyS = KY[sel]
    kxS = KX[sel]

    r_max = np.clip(np.float32(sigma_c), 0.01, 0.9) * np.float32(FC)
    n_r = int(np.sqrt(N_SOURCE * 0.3)) + 1
    n_theta = int(N_SOURCE / n_r) + 1
    r = np.linspace(0.0, 1.0, n_r, dtype=np.float32) * r_max
    theta = np.linspace(0.0, 2.0 * PI, n_theta, dtype=np.float32)
    rr, tt = np.meshgrid(r, theta, indexing="xy")
    fs = np.stack([(rr * np.cos(tt)).ravel(), (rr * np.sin(tt)).ravel()],
                  axis=1)[:N_SOURCE].astype(np.float32)

    fyS = (kyS / float(H * DX)).astype(np.float32)
    fxS = (kxS / float(W * DX)).astype(np.float32)
    cols = []
    for fp in fs:
        f1 = np.hypot(fxS + np.float32(fp[0] / 2), fyS + np.float32(fp[1] / 2))
        f2 = np.hypot(fxS - np.float32(fp[0] / 2), fyS - np.float32(fp[1] / 2))
        cols.append(((f1 <= np.float32(FC)) & (f2 <= np.float32(FC)))
                    .astype(np.float64))
    MS = np.stack(cols, axis=1)
    G = MS.T @ MS
    w_, V_ = np.linalg.eigh(G)
    idx = np.argsort(w_)[::-1]
    w_ = np.maximum(w_[idx], 0.0)
    V_ = V_[:, idx]
    keep = [k for k in range(min(NK, N_SOCS)) if w_[k] > 1e-9 * w_[0]]
    alpha = w_[keep]
    US = MS @ V_[:, keep] / np.sqrt(alpha)

    dy = np.arange(NSUP) - HS
    Ay = np.exp(2j * PI * np.outer(dy, kyS) / H) * ((-1.0) ** dy)[:, None]
    Ax = np.exp(2j * PI * np.outer(dy, kxS) / W) * ((-1.0) ** dy)[:, None]
    kerns = np.einsum("ys,sk,xs->kyx", Ay, US, Ax, optimize=True).real
    return kerns * (SCL * np.sqrt(alpha)[:, None, None] / (H * W))


# ---------------- host: stationaries ----------------

def _pack_consts():
    h1 = np.arange(128)[:, None]
    k1 = np.arange(128)[None, :]
    SA = [np.exp(-2j * PI * (h1 * k1 / 128.0 + h2 * k1 / 1024.0))
          for h2 in range(8)]
    a = (np.arange(128) // 8)[:, None]
    b = (np.arange(128) % 8)[:, None]
    c = (np.arange(128) // 8)[None, :]
    d = (np.arange(128) % 8)[None, :]
    M34 = np.exp(-2j * PI * (a * c / 16.0 + b * d / 8.0))
    # permutation: new index q = h2*16 + w1  <->  old index w1*8 + h2
    qperm = np.array([(q % 16) * 8 + q // 16 for q in range(128)])
    M34p = M34[qperm, :]      # fwd: rows = q-order, cols = (kw1*8+kh2)
    M34i = M34[:, qperm]      # inv: rows = (kw1*8+kh2), cols = q-order
    w2 = np.arange(128)[:, None]
    kw2 = np.arange(128)[None, :]
    SB = [np.exp(-2j * PI * (w2 * kw2 / 128.0 + w2 * kw1 / 2048.0))
          for kw1 in range(16)]
    IA = [np.conj(m).T for m in SA]
    IB = [np.conj(m).T for m in SB]

    dy = np.arange(NSUP) - HS
    Ah = []
    for bb in range(8):
        kh = np.arange(128)[None, :] + 128 * bb
        Ah.append(np.exp(-2j * PI * dy[:, None] * kh / 1024.0))
    Aw = []
    for kw1 in range(16):
        kw = kw1 + 16 * np.arange(128)[None, :]
        Aw.append(np.exp(-2j * PI * dy[:, None] * kw / 2048.0))

    def planes(m, n_planes):
        m64 = np.asarray(m, np.complex128)
        out = []
        for pm in (m64.real, m64.imag, -m64.imag)[:n_planes]:
            pm = np.asarray(pm, np.float32)
            if pm.shape[0] < 128:
                z = np.zeros((128, 128), np.float32)
                z[:pm.shape[0]] = pm
                pm = z
            out.append(pm)
        return out

    fwd = ([p for m in SA for p in planes(m, 2)]
           + planes(M34p, 3)
           + [p for m in SB for p in planes(m, 3)]
           + [p for m in Aw for p in planes(m, 3)])
    inv = ([p for m in IA for p in planes(m, 3)]
           + planes(M34i, 3)
           + [p for m in IB for p in planes(m, 3)])
    cf16 = np.concatenate(fwd, axis=1).astype(np.float16)
    ci16 = np.concatenate(inv, axis=1).astype(np.float16)
    assert cf16.shape[1] == NC16_COLS and ci16.shape[1] == NI16_COLS

    c32 = np.concatenate([p for m in Ah for p in planes(m, 2)],
                         axis=1).astype(np.float32)
    assert c32.shape[1] == NC32_COLS
    return cf16, ci16, c32


# ---------------- host: input packing ----------------

def _mask_layout(mask):
    """[p=h1 | h2*2048 + w2*16 + w1] fp16, rolled and scaled by 1/SCL."""
    m_u = np.roll(np.asarray(mask, np.float32), (-H // 2, -W // 2), axis=(0, 1))
    m_u = m_u * np.float32(1.0 / SCL)
    m4 = m_u.reshape(P, 8, 16, 128).transpose(0, 1, 3, 2)  # [h1,h2,w2,w1]
    return np.ascontiguousarray(m4.reshape(P, FREE)).astype(np.float16)


def _kern_pack(kp):
    """complex (114,114) crop-indexed pair -> [128, 342] fp32
    layout [-im | re | im] (step1 rhs trick), pad rows zero."""
    q = np.zeros((P, 3 * NSUP), np.float32)
    q[:NSUP, 0:NSUP] = -kp.imag
    q[:NSUP, NSUP:2 * NSUP] = kp.real
    q[:NSUP, 2 * NSUP:3 * NSUP] = kp.imag
    return q


# ---------------- bass program ----------------

_NC_CACHE = {}


def _build_nc(num_devices=N_CORES):
    import concourse.bacc as bacc
    import concourse.mybir as mybir
    import concourse.tile as tile

    dt = mybir.dt.float32
    db = mybir.dt.float16
    nc = bacc.Bacc("TRN2", target_bir_lowering=False, debug=False,
                   num_devices=num_devices)
    mask_d = nc.dram_tensor("mask_l", [P, FREE], db, kind="ExternalInput")
    kq_d = nc.dram_tensor("kq", [P, 3 * NSUP], dt, kind="ExternalInput")
    cf16_d = nc.dram_tensor("cf16", [P, NC16_COLS], db, kind="ExternalInput")
    ci16_d = nc.dram_tensor("ci16", [P, NI16_COLS], db, kind="ExternalInput")
    c32_d = nc.dram_tensor("c32", [P, NC32_COLS], dt, kind="ExternalInput")
    out_d = nc.dram_tensor("acc_out", [P, FREE], dt, kind="ExternalOutput")

    with tile.TileContext(nc) as tc:
        with (
            tc.tile_pool(name="img", bufs=1) as img_pool,
            tc.tile_pool(name="consts", bufs=1) as const_pool,
            tc.tile_pool(name="ksm", bufs=1) as ksm_pool,
            tc.tile_pool(name="mstream", bufs=2) as ms_pool,
            tc.tile_pool(name="acc", bufs=3) as acc_pool,
            tc.tile_pool(name="tmp", bufs=4) as tmp_pool,
            tc.tile_pool(name="t32", bufs=4) as t32_pool,
            tc.tile_pool(name="ps", bufs=8, space="PSUM") as ps_pool,
        ):
            xr = img_pool.tile([P, FREE], db, tag="xr")
            xi = img_pool.tile([P, FREE], db, tag="xi")
            yr = img_pool.tile([P, FREE], db, tag="yr")
            yi = img_pool.tile([P, FREE], db, tag="yi")
            c16 = const_pool.tile([P, NC16_COLS], db, tag="c16")
            c32 = const_pool.tile([P, NC32_COLS], dt, tag="c32")
            kt = ksm_pool.tile([P, 3 * NSUP], dt, tag="kt")
            tk1 = ksm_pool.tile([P, 2 * 8 * 128], db, tag="tk1")
            tk2 = ksm_pool.tile([P, 2 * 8 * 128], db, tag="tk2")

            A = (xr, xi)
            B = (yr, yi)

            def C16(off):
                return c16[:, off:off + 128]

            def C32(off):
                return c32[:, off:off + 128]

            _cp = [0]

            def copy_out(dst, src):
                # PSUM evacuation: only VectorE/ScalarE may read PSUM
                i = _cp[0] = _cp[0] + 1
                eng = (nc.vector.tensor_copy, nc.scalar.copy)[i % 2]
                eng(dst, src)

            nc.sync.dma_start(c16[:, 0:16 * 128], cf16_d.ap()[:, 0:16 * 128])
            nc.sync.dma_start(c16[:, 16 * 128:NC16_COLS],
                              cf16_d.ap()[:, 16 * 128:NC16_COLS])
            nc.sync.dma_start(c32[:], c32_d.ap())
            nc.sync.dma_start(kt[:], kq_d.ap())
            nc.gpsimd.memset(tk1[:], 0.0)

            # ---- S1: contract h1 per h2 (SA fp16) ----
            # mask chunk (h2, cw): cols (w2 in {64cw..}, w1:16), contiguous
            # dst X1 (A): free = w2*128 + h2*16 + w1: 32B-contig w1-runs
            xrv = xr[:].rearrange("p (w2 h2 w1) -> p w2 h2 w1",
                                  w2=128, h2=8, w1=16)
            xiv = xi[:].rearrange("p (w2 h2 w1) -> p w2 h2 w1",
                                  w2=128, h2=8, w1=16)
            for cw in range(2):
                for h2 in range(8):
                    mt = ms_pool.tile([P, 1024], db, tag="ms")
                    nc.sync.dma_start(
                        mt[:], mask_d.ap()[:, h2 * 2048 + cw * 1024:
                                           h2 * 2048 + (cw + 1) * 1024])
                    ps4 = [ps_pool.tile([P, 512], dt, tag="ps",
                                        name=f"s1{i}") for i in range(4)]
                    for h in range(2):
                        nc.tensor.matmul(ps4[h][:], C16(F_SA(h2, RE)),
                                         mt[:, h * 512:(h + 1) * 512],
                                         start=True, stop=True)
                    for h in range(2):
                        nc.tensor.matmul(ps4[2 + h][:], C16(F_SA(h2, IM)),
                                         mt[:, h * 512:(h + 1) * 512],
                                         start=True, stop=True)
                    for h in range(2):
                        psr = ps4[h][:].rearrange("p (a b) -> p a b",
                                                  a=32, b=16)
                        psi = ps4[2 + h][:].rearrange("p (a b) -> p a b",
                                                      a=32, b=16)
                        w0 = 64 * cw + 32 * h
                        copy_out(xrv[:, w0:w0 + 32, h2, :], psr)
                        copy_out(xiv[:, w0:w0 + 32, h2, :], psi)

            # ---- XBAR pass helper: per-128x128-tile transposes,
            # re plane on the SP queue, im plane on the Act queue ----
            def xbar_pass(src, dst, n_sub=4):
                # interleave planes so dependent stages unblock after ~2 subs
                step = 128 // n_sub
                for s in range(n_sub):
                    for pl in range(2):
                        dv = dst[pl][:].rearrange("p (j i) -> p j i",
                                                  j=128, i=128)
                        j0, j1 = s * step, (s + 1) * step
                        nc.sync.dma_start(
                            dv[:, j0:j1, :],
                            src[pl][:, j0 * 128:j1 * 128],
                            transpose=True)

            # ---- T1: A -> B : [kh1, q]@w2 -> [q, kh1]@w2 ----
            xbar_pass(A, B)

            # ---- generic full complex stage: 16 groups x 2 halves of 512,
            # both halves of a group share the group's stationaries ----
            def full_stage(src, dst, mats, rhs_of, dst_of, conj=False,
                           cp=None):
                cp = cp or copy_out
                for g in range(16):
                    mre, mim, mimn = mats(g)
                    if conj:
                        mim, mimn = mimn, mim
                    rre = [rhs_of(src[0], g, h) for h in range(2)]
                    rim = [rhs_of(src[1], g, h) for h in range(2)]
                    pre = [ps_pool.tile([P, 512], dt, tag="ps",
                                        name=f"fre{h}") for h in range(2)]
                    pim = [ps_pool.tile([P, 512], dt, tag="ps",
                                        name=f"fim{h}") for h in range(2)]
                    for h in range(2):
                        nc.tensor.matmul(pre[h][:], C16(mre), rre[h],
                                         start=True, stop=False)
                    for h in range(2):
                        nc.tensor.matmul(pim[h][:], C16(mre), rim[h],
                                         start=True, stop=False)
                    for h in range(2):
                        nc.tensor.matmul(pre[h][:], C16(mimn), rim[h],
                                         start=False, stop=True)
                    for h in range(2):
                        nc.tensor.matmul(pim[h][:], C16(mim), rre[h],
                                         start=False, stop=True)
                    for h in range(2):
                        cp(dst_of(dst[0], g, h), pre[h][:])
                        cp(dst_of(dst[1], g, h), pim[h][:])

            # S3'/I2-style: rhs contiguous (outer:4, inner:128); the
            # transposing scatter happens in the PSUM evacuation instead
            def contig_rhs(pap, g, h):
                off = g * 1024 + h * 512
                return pap[:, off:off + 512]

            def swap_dst(pap, g, h):
                # psum cols (a:4, b:128) -> dst free = b*128 + (4cc + a)
                v = pap[:].rearrange("p (b a) -> p b a", b=128, a=128)
                cc = 2 * g + h
                return v[:, :, 4 * cc:4 * cc + 4].transpose([0, 2, 1])

            contig_dst = contig_rhs

            # S4'/step2/I1-style: per-kw1 (kh1:64-half, kh2:8) slices
            def dkq_rhs(pap, kw1, h):
                v = pap[:].rearrange("p (k q d) -> p k q d", k=128, q=16, d=8)
                return v[:, 64 * h:64 * h + 64, kw1, :]

            dkq_dst = dkq_rhs

            # ---- S3': contract q via M34p : B -> A ----
            full_stage(B, A, lambda cc: (F_M34(RE), F_M34(IM), F_M34(IMN)),
                       contig_rhs, swap_dst)

            # ---- T2: A -> B : [d, w2]@kh1 -> [w2, d]@kh1 ----
            # n_sub=2: sub 0 covers exactly the kh1<64 half S4' h=0 needs
            xbar_pass(A, B, n_sub=2)

            # ---- S4': contract w2 per kw1 (SB) : B -> A (mask spectrum)
            # single-chunk groups (kw1, h) so S4' starts after half of T2 ----
            def full_stage1(src, dst, mats, rhs_of, dst_of):
                for h in range(2):
                    for g in range(16):
                        mre, mim, mimn = mats(g)
                        rre = rhs_of(src[0], g, h)
                        rim = rhs_of(src[1], g, h)
                        pre = ps_pool.tile([P, 512], dt, tag="ps", name="g1re")
                        pim = ps_pool.tile([P, 512], dt, tag="ps", name="g1im")
                        nc.tensor.matmul(pre[:], C16(mre), rre,
                                         start=True, stop=False)
                        nc.tensor.matmul(pim[:], C16(mre), rim,
                                         start=True, stop=False)
                        nc.tensor.matmul(pre[:], C16(mimn), rim,
                                         start=False, stop=True)
                        nc.tensor.matmul(pim[:], C16(mim), rre,
                                         start=False, stop=True)
                        copy_out(dst_of(dst[0], g, h), pre[:])
                        copy_out(dst_of(dst[1], g, h), pim[:])

            full_stage1(B, A, lambda kw1: (F_SB(kw1, RE), F_SB(kw1, IM),
                                           F_SB(kw1, IMN)),
                        dkq_rhs, contig_dst)

            # ---- kernel fwd FFT step1 (fp32, contract h) ----
            for bb in range(8):
                ps = ps_pool.tile([P, 512], dt, tag="ps", name="k1")
                nc.tensor.matmul(ps[:, 0:2 * NSUP], C32((bb * 2 + RE) * 128),
                                 kt[:, NSUP:3 * NSUP], start=True, stop=False)
                nc.tensor.matmul(ps[:, 0:2 * NSUP], C32((bb * 2 + IM) * 128),
                                 kt[:, 0:2 * NSUP], start=False, stop=True)
                copy_out(tk1[:, bb * 128:bb * 128 + NSUP], ps[:, 0:NSUP])
                copy_out(tk1[:, (8 + bb) * 128:(8 + bb) * 128 + NSUP],
                         ps[:, NSUP:2 * NSUP])

            # Tk xbar: [kh1, (pl,b)*128+w] -> tk2 [w | (pl,b)*128 + kh1]
            tk2v = tk2[:].rearrange("p (j i) -> p j i", j=16, i=128)
            nc.sync.dma_start(tk2v[:, :, :], tk1[:, :], transpose=True)

            # ---- step2: contract w per kw1 (Aw) -> B (kernel spectrum) ----
            # rhs contiguous tk2 plane halves; psum (kh2r:4, kh1:128)
            # scattered into the F layout by the evacuation
            def k2_dst(pap, kw1, h):
                v = pap[:].rearrange("p (q k d) -> p q k d", q=16, k=128, d=8)
                return v[:, kw1, :, 4 * h:4 * h + 4].transpose([0, 2, 1])

            def s2_copy_out(dst, src_):
                i = _cp[0] = _cp[0] + 1
                eng = (nc.scalar.copy, nc.scalar.copy,
                       nc.vector.tensor_copy)[i % 3]
                eng(dst, src_)

            full_stage((0, 1), B,
                       lambda kw1: (F_AW(kw1, RE), F_AW(kw1, IM),
                                    F_AW(kw1, IMN)),
                       lambda pl, g, h: tk2[:, pl * 1024 + h * 512:
                                            pl * 1024 + (h + 1) * 512],
                       k2_dst, cp=s2_copy_out)

            # ---- swap consts to inv set, split so I1 (IB region) does
            # not wait on the Aw-tail stationary reads of step2 ----
            nc.sync.dma_start(c16[:, 0:27 * 128], ci16_d.ap()[:, 0:27 * 128])
            nc.sync.dma_start(c16[:, 27 * 128:NI16_COLS],
                              ci16_d.ap()[:, 27 * 128:NI16_COLS])

            # ---- product: B = A * B (complex fp16, in place) ----
            for cc in range(16):
                sl = slice(cc * 1024, (cc + 1) * 1024)
                t0 = tmp_pool.tile([P, 1024], db, tag="tp")
                t1 = tmp_pool.tile([P, 1024], db, tag="tp")
                t2 = tmp_pool.tile([P, 1024], db, tag="tp")
                t3 = tmp_pool.tile([P, 1024], db, tag="tp")
                # gpsimd TENSOR_TENSOR is ~2.4x slower than DVE: 2 ops vs 4
                nc.vector.tensor_mul(t0[:], xr[:, sl], yr[:, sl])
                nc.gpsimd.tensor_mul(t1[:], xi[:, sl], yi[:, sl])
                nc.vector.tensor_mul(t2[:], xr[:, sl], yi[:, sl])
                nc.gpsimd.tensor_mul(t3[:], xi[:, sl], yr[:, sl])
                nc.vector.tensor_sub(yr[:, sl], t0[:], t1[:])
                nc.vector.tensor_add(yi[:, sl], t2[:], t3[:])

            # ---- I1: contract kw2 per kw1 (IB) : B -> A ----
            # evacs biased to ScalarE: VectorE is saturated by the product
            _cp_sav = _cp[0]
            _cp[0] = 1  # start so scalar gets first pick

            def scalar_copy_out(dst, src_):
                i = _cp[0] = _cp[0] + 1
                eng = (nc.scalar.copy, nc.scalar.copy, nc.scalar.copy,
                       nc.vector.tensor_copy)[i % 4]
                eng(dst, src_)

            full_stage(B, A, lambda kw1: (I_IB(kw1, RE), I_IB(kw1, IM),
                                          I_IB(kw1, IMN)),
                       lambda pap, g, h: contig_dst(pap, g, h),
                       dkq_dst, cp=scalar_copy_out)

            # ---- T1': A -> B : [w2, d]@kh1 -> [d, w2]@kh1 ----
            xbar_pass(A, B)

            # ---- I2: contract d via M34i (conj) : B -> A ----
            full_stage(B, A, lambda cc: (I_M34(RE), I_M34(IM), I_M34(IMN)),
                       contig_rhs, swap_dst, conj=True)

            # ---- T2': A -> B : [q, kh1]@w2 -> [kh1, q]@w2 ----
            # n_sub=2: sub 0 covers exactly the w2<64 half I3 needs first
            xbar_pass(A, B, n_sub=2)

            # ---- I3: contract kh1 per h2 (IA), square, accumulate, out ----
            yrv = yr[:].rearrange("p (w2 h2 w1) -> p w2 h2 w1",
                                  w2=128, h2=8, w1=16)
            yiv = yi[:].rearrange("p (w2 h2 w1) -> p w2 h2 w1",
                                  w2=128, h2=8, w1=16)
            for h2 in range(8):
              for half in range(2):
                arow = acc_pool.tile([P, 1024], dt, tag="arow")
                arv = arow[:].rearrange("p (w1 w2) -> p w1 w2", w1=16, w2=64)
                for c in (2 * half, 2 * half + 1):
                    # rhs enum (w2r:32, w1:16): w1 innermost = 32B-contig runs
                    rre = yrv[:, 32 * c:32 * c + 32, h2, :]
                    rim = yiv[:, 32 * c:32 * c + 32, h2, :]
                    pre = ps_pool.tile([P, 512], dt, tag="ps", name="i3re")
                    pim = ps_pool.tile([P, 512], dt, tag="ps", name="i3im")
                    nc.tensor.matmul(pre[:], C16(I_IA(h2, RE)), rre,
                                     start=True, stop=False)
                    nc.tensor.matmul(pim[:], C16(I_IA(h2, RE)), rim,
                                     start=True, stop=False)
                    nc.tensor.matmul(pre[:], C16(I_IA(h2, IMN)), rim,
                                     start=False, stop=True)
                    nc.tensor.matmul(pim[:], C16(I_IA(h2, IM)), rre,
                                     start=False, stop=True)
                    # psum cols (w1:16, w2r:32); squares into contig temps
                    sre = t32_pool.tile([P, 512], dt, tag="sq")
                    sim_ = t32_pool.tile([P, 512], dt, tag="sq")
                    nc.scalar.square(sre[:], pre[:])
                    nc.scalar.square(sim_[:], pim[:])
                    srev = sre[:].rearrange("p (a b) -> p a b", a=32, b=16)
                    simv = sim_[:].rearrange("p (a b) -> p a b", a=32, b=16)
                    c0 = 32 * (c - 2 * half)
                    nc.vector.tensor_add(
                        arv[:, :, c0:c0 + 32].transpose([0, 2, 1]),
                        srev, simv)
                odv = out_d.ap()[:, h2 * 2048:(h2 + 1) * 2048].rearrange(
                    "p (w1 w2) -> p w1 w2", w1=16, w2=128)
                nc.sync.dma_start(odv[:, :, 64 * half:64 * half + 64],
                                  arow[:].rearrange("p (a b) -> p a b",
                                                    a=16, b=64))

    nc.compile()
    return nc


# ---------------- entry point ----------------

def _prepare_inputs(mask, sigma_c):
    mask = np.asarray(mask, np.float32)
    kerns = _compute_kernels(float(np.asarray(sigma_c)))
    assert len(kerns) == NK
    mask_l = _mask_layout(mask)
    cf16, ci16, c32 = _pack_consts()
    in_maps = []
    for c in range(N_CORES):
        kp = kerns[c] + 1j * kerns[c + 8]
        in_maps.append({
            "mask_l": mask_l,
            "kq": _kern_pack(kp),
            "cf16": cf16,
            "ci16": ci16,
            "c32": c32,
        })
    return in_maps


def _combine(results):
    acc = np.zeros((H, W), np.float64)
    for c in range(N_CORES):
        acc += results[c]["acc_out"].astype(np.float64).reshape(H, W)
    I = np.fft.fftshift(acc)
    return (I / I.max()).astype(np.float32)


def kernel(mask, sigma_c, defocus_z4):
    from concourse import bass_utils

    in_maps = _prepare_inputs(mask, sigma_c)
    if "nc" not in _NC_CACHE:
        _NC_CACHE["nc"] = _build_nc()
    nc = _NC_CACHE["nc"]
    res = bass_utils.run_bass_kernel_spmd(nc, in_maps,
                                          core_ids=list(range(N_CORES)))
    return _combine(res.results)
